# revision 25
# baseline (speedup 1.0000x reference)
"""Dense dot-product attention on 8 Trainium2 NeuronCores.

Problem: query/key/value [32, 2048, 64] fp32 -> softmax(Q K^T / 8) V.
Sharding: batch dim split 4-per-core across 8 cores (data parallel, no
collectives). Each core computes full attention for its 4 batches.

All matmuls run in fp16: 1 cycle/column on the PE, and 16-bit-class
matmuls are the only ones the PE's HAM clock-gate counts as activity
(an fp32/f32r-only kernel is stuck at 1.2 GHz; a dense fp16 stream
keeps the array at 2.4 GHz). fp16 has a 11-bit mantissa; with d=64 and
unit-normal inputs the score error is ~5e-4 std and the final output
lands within ~1e-3 of the fp32 reference.

Per-batch dataflow:
  1. DMA Q,K natural [2048,64]; DVE-cast to fp16; PE-transpose 128-row
     tiles -> [64,2048] fp16 in SBUF; DMA-duplicate into both partition
     halves for row-packed matmul pairs.
  2. S^T[k,q] = Kh^T.T @ Qh^T, two k-tiles concurrently (row strips
     0-63 / 64-127), into fp32 PSUM [128k, 1024q] blocks.
  3. exp on ScalarE straight out of PSUM (scale=1/8 folded in), fp16
     out. No max-subtraction: scores ~ N(0,1), exp cannot overflow.
  4. P@V via fp16 matmul with lhsT = [V | ones] [128k, 65]: accumulates
     out^T [65, q] in fp32 PSUM over the 16 k-tiles; row 64 = softmax
     denominator.
  5. PE-transpose out^T chunks -> [128q, 65], DVE reciprocal of col 64,
     row-scale cols 0..63, DMA out.

The next batch's input transposes are interleaved into the current
batch's matmul stream so the PE and ScalarE never drain between batches.
"""

import numpy as np

B, L, D = 32, 2048, 64
NCORES = 8
B_SH = B // NCORES          # 4 batches per core
LT = L // 128               # 16 k/l tiles of 128
NQH = 2                     # q processed in halves of 1024
QHW = L // NQH              # 1024
SCALE = 1.0 / np.sqrt(np.float32(D))  # 0.125

_cached = {}


def _build():
    import concourse.bacc as bacc
    import concourse.tile as tile
    from concourse import mybir
    from concourse.masks import make_identity

    f32 = mybir.dt.float32
    fp16 = mybir.dt.float16
    Exp = mybir.ActivationFunctionType.Exp

    nc = bacc.Bacc("TRN2", target_bir_lowering=False, debug=False)

    q_d = nc.dram_tensor("query", [B_SH, L, D], f32, kind="ExternalInput")
    k_d = nc.dram_tensor("key", [B_SH, L, D], f32, kind="ExternalInput")
    v_d = nc.dram_tensor("value", [B_SH, L, D], f32, kind="ExternalInput")
    o_d = nc.dram_tensor("out", [B_SH, L, D], f32, kind="ExternalOutput")

    with tile.TileContext(nc) as tc:
        with (
            tc.tile_pool(name="consts", bufs=1) as consts,
            tc.tile_pool(name="nat", bufs=2) as nat,
            tc.tile_pool(name="nath", bufs=2) as nath,
            tc.tile_pool(name="vst", bufs=2) as vst,
            tc.tile_pool(name="qkt", bufs=2) as qkt,
            tc.tile_pool(name="vr", bufs=2) as vrp,
            tc.tile_pool(name="er", bufs=3) as erp,
            tc.tile_pool(name="pvsb", bufs=2) as pvsb,
            tc.tile_pool(name="oall", bufs=2) as oallp,
            tc.tile_pool(name="rz", bufs=4) as rzp,
            tc.tile_pool(name="sps", bufs=2, space="PSUM") as sps,
            tc.tile_pool(name="pvps", bufs=1, space="PSUM") as pvps,
            tc.tile_pool(name="trps", bufs=2, space="PSUM") as trps,
        ):
            ident = consts.tile([128, 128], f32)
            make_identity(nc, ident)
            identh = consts.tile([128, 128], fp16)
            nc.vector.tensor_copy(out=identh, in_=ident)
            wsrc = consts.tile([128, 512], fp16)
            nc.vector.memset(wsrc, 1.0)

            def warmer(n=512):
                wt = trps.tile([64, 512], f32, tag="tr")
                nc.tensor.matmul(wt[:, 0:n], wsrc[:, 0:64], wsrc[:, 0:n],
                                 start=True, stop=True, skip_group_check=True)

            # per-batch persistent tiles
            qkT = {}   # b -> (qhT, khT) [128, 2048] fp16, halves identical
            v_r = {}   # b -> [128, 16, 65] fp16  (col 64 = 1.0)

            def prep_load(b):
                """DMA loads + fp16 casts + transpose jobs for batch b."""
                q_nat = nat.tile([128, LT, D], f32, tag="qnat")
                k_nat = nat.tile([128, LT, D], f32, tag="knat")
                nc.sync.dma_start(
                    out=q_nat, in_=q_d.ap()[b].rearrange("(t p) d -> p t d", p=128))
                nc.sync.dma_start(
                    out=k_nat, in_=k_d.ap()[b].rearrange("(t p) d -> p t d", p=128))

                qh_nat = nath.tile([128, LT, D], fp16, tag="qh_nat")
                kh_nat = nath.tile([128, LT, D], fp16, tag="kh_nat")
                nc.vector.tensor_copy(out=qh_nat, in_=q_nat)
                nc.vector.tensor_copy(out=kh_nat, in_=k_nat)

                qhT = qkt.tile([128, L], fp16, tag="qhT")
                khT = qkt.tile([128, L], fp16, tag="khT")

                v_stage = vst.tile([128, LT, D], f32, tag="vstage")
                nc.sync.dma_start(
                    out=v_stage, in_=v_d.ap()[b].rearrange("(t p) d -> p t d", p=128))
                vr = vrp.tile([128, LT, D + 1], fp16, tag="vr")
                nc.vector.tensor_copy(out=vr[:, :, 0:D], in_=v_stage)
                nc.vector.memset(vr[:, :, D:D + 1], 1.0)

                qkT[b] = (qhT, khT)
                v_r[b] = vr

                jobs = []
                for lt in range(LT):
                    for src, dst in ((qh_nat, qhT), (kh_nat, khT)):
                        def tr_job(src=src, dst=dst, lt=lt):
                            tp = trps.tile([64, 128], fp16, tag="tr")
                            nc.tensor.transpose(tp, src[:, lt, :], identh)
                            nc.vector.tensor_copy(
                                out=dst[0:64, lt * 128:(lt + 1) * 128], in_=tp)
                        jobs.append(tr_job)

                def dup_job(t):
                    def job():
                        nc.sync.dma_start(out=t[64:128, :], in_=t[0:64, :])
                    return job

                jobs.append(dup_job(qhT))
                jobs.append(dup_job(khT))
                return jobs

            pending = []   # deferred small jobs woven into the MM stream

            def main(b, next_jobs):
                qhT, khT = qkT.pop(b)
                vr = v_r.pop(b)
                slot = 0
                for qh in range(NQH):
                    q0 = qh * QHW
                    pv = pvps.tile([D + 1, QHW], f32, tag="pv")
                    for kp in range(LT // 2):      # pairs of k-tiles
                        ka, kb = 2 * kp, 2 * kp + 1
                        # interleave deferred out-work + next batch's prep
                        for _ in range(2):
                            if pending:
                                pending.pop(0)()
                        for _ in range(2):
                            if slot < len(next_jobs):
                                next_jobs[slot]()
                                slot += 1
                        s_a = sps.tile([128, QHW], f32, tag="s")
                        s_b = sps.tile([128, QHW], f32, tag="s")
                        ha, hb = slice(0, 64), slice(64, 128)
                        ksa = slice(ka * 128, (ka + 1) * 128)
                        ksb = slice(kb * 128, (kb + 1) * 128)
                        for j in range(QHW // 512):
                            js = slice(j * 512, (j + 1) * 512)
                            qs = slice(q0 + j * 512, q0 + (j + 1) * 512)
                            nc.tensor.matmul(
                                s_a[:, js], khT[ha, ksa], qhT[ha, qs],
                                start=True, stop=True)
                            nc.tensor.matmul(
                                s_b[:, js], khT[hb, ksb], qhT[hb, qs],
                                start=True, stop=True)
                        for kt, s_ps in ((ka, s_a), (kb, s_b)):
                            e_r = erp.tile([128, QHW], fp16, tag="e")
                            nc.scalar.activation(out=e_r, in_=s_ps, func=Exp,
                                                 scale=float(SCALE))
                            for j in range(QHW // 512):
                                js = slice(j * 512, (j + 1) * 512)
                                nc.tensor.matmul(
                                    pv[:, js], vr[:, kt, :], e_r[:, js],
                                    start=(kt == 0), stop=(kt == LT - 1))

                    # defer psum evacuation + out-transpose + normalize:
                    # woven into subsequent pair-slots so the in-order PE
                    # stream never blocks on this at the qh boundary
                    pv_sb = pvsb.tile([D + 1, QHW], f32, tag="pvsb")
                    o_all = oallp.tile([128, QHW // 128, D], f32, tag="oall")

                    def evac_job(pv=pv, pv_sb=pv_sb):
                        nc.vector.tensor_copy(out=pv_sb, in_=pv)
                    pending.append(evac_job)

                    for qt in range(QHW // 128):
                        def out_job(qt=qt, pv_sb=pv_sb, o_all=o_all):
                            ot = trps.tile([128, D + 1], f32, tag="tr")
                            nc.tensor.transpose(
                                ot, pv_sb[:, qt * 128:(qt + 1) * 128],
                                ident[0:D + 1, 0:D + 1])
                            rz = rzp.tile([128, 1], f32, tag="rz")
                            nc.vector.reciprocal(out=rz, in_=ot[:, D:D + 1])
                            nc.vector.tensor_scalar_mul(
                                out=o_all[:, qt, :], in0=ot[:, 0:D],
                                scalar1=rz)
                        pending.append(out_job)

                    def store_job(b=b, q0=q0, o_all=o_all):
                        nc.sync.dma_start(
                            out=o_d.ap()[b, q0:q0 + QHW, :].rearrange(
                                "(t p) d -> p t d", p=128),
                            in_=o_all)
                    pending.append(store_job)
                while slot < len(next_jobs):
                    next_jobs[slot]()
                    slot += 1

            for _ in range(24):
                warmer()
            jobs0 = prep_load(0)
            for job in jobs0:
                job()
                warmer()
            for b in range(B_SH):
                nxt = prep_load(b + 1) if b + 1 < B_SH else []
                main(b, nxt)
            for job in pending:
                job()

    nc.finalize()
    return nc


def _get_nc():
    if "nc" not in _cached:
        _cached["nc"] = _build()
    return _cached["nc"]


def kernel(query, key, value):
    from concourse.bass_utils import run_bass_kernel_spmd

    nc = _get_nc()
    query = np.ascontiguousarray(query, dtype=np.float32)
    key = np.ascontiguousarray(key, dtype=np.float32)
    value = np.ascontiguousarray(value, dtype=np.float32)

    in_maps = []
    for c in range(NCORES):
        sl = slice(c * B_SH, (c + 1) * B_SH)
        in_maps.append({
            "query": query[sl], "key": key[sl], "value": value[sl]})

    res = run_bass_kernel_spmd(nc, in_maps, core_ids=list(range(NCORES)))
    out = np.concatenate([r["out"] for r in res.results], axis=0)
    return out


# revision 26
# speedup vs baseline: 1.1344x; 1.1344x over previous
"""Dense dot-product attention on 8 Trainium2 NeuronCores.

Problem: query/key/value [32, 2048, 64] fp32 -> softmax(Q K^T / 8) V.
Sharding: batch dim split 4-per-core across 8 cores (data parallel, no
collectives). Each core computes full attention for its 4 batches.

All matmuls run in fp16: 1 cycle/column on the PE, and 16-bit-class
matmuls are the only ones the PE's HAM clock-gate counts as activity
(an fp32/f32r-only kernel is stuck at 1.2 GHz; a dense fp16 stream
keeps the array at 2.4 GHz). fp16 has a 11-bit mantissa; with d=64 and
unit-normal inputs the score error is ~5e-4 std and the final output
lands within ~1e-3 of the fp32 reference.

Per-batch dataflow:
  1. DMA Q,K natural [2048,64]; DVE-cast to fp16; PE-transpose 128-row
     tiles -> [64,2048] fp16 in SBUF; DMA-duplicate into both partition
     halves for row-packed matmul pairs.
  2. S^T[k,q] = Kh^T.T @ Qh^T, two k-tiles concurrently (row strips
     0-63 / 64-127), into fp32 PSUM [128k, 1024q] blocks.
  3. exp on ScalarE straight out of PSUM (scale=1/8 folded in), fp16
     out. No max-subtraction: scores ~ N(0,1), exp cannot overflow.
  4. P@V via fp16 matmul with lhsT = [V | ones] [128k, 65]: accumulates
     out^T [65, q] in fp32 PSUM over the 16 k-tiles; row 64 = softmax
     denominator.
  5. PE-transpose out^T chunks -> [128q, 65], DVE reciprocal of col 64,
     row-scale cols 0..63, DMA out.

The next batch's input transposes are interleaved into the current
batch's matmul stream so the PE and ScalarE never drain between batches.
"""

import numpy as np

B, L, D = 32, 2048, 64
NCORES = 8
B_SH = B // NCORES          # 4 batches per core
LT = L // 128               # 16 k/l tiles of 128
NQH = 2                     # q processed in halves of 1024
QHW = L // NQH              # 1024
SCALE = 1.0 / np.sqrt(np.float32(D))  # 0.125

_cached = {}


def _build():
    import concourse.bacc as bacc
    import concourse.tile as tile
    from concourse import mybir
    from concourse.masks import make_identity

    f32 = mybir.dt.float32
    fp16 = mybir.dt.float16
    Exp = mybir.ActivationFunctionType.Exp

    nc = bacc.Bacc("TRN2", target_bir_lowering=False, debug=False)

    q_d = nc.dram_tensor("query", [B_SH, L, D], f32, kind="ExternalInput")
    k_d = nc.dram_tensor("key", [B_SH, L, D], f32, kind="ExternalInput")
    v_d = nc.dram_tensor("value", [B_SH, L, D], f32, kind="ExternalInput")
    o_d = nc.dram_tensor("out", [B_SH, L, D], f32, kind="ExternalOutput")

    with tile.TileContext(nc) as tc:
        with (
            tc.tile_pool(name="consts", bufs=1) as consts,
            tc.tile_pool(name="nat", bufs=2) as nat,
            tc.tile_pool(name="nath", bufs=2) as nath,
            tc.tile_pool(name="vst", bufs=2) as vst,
            tc.tile_pool(name="qkt", bufs=2) as qkt,
            tc.tile_pool(name="vr", bufs=2) as vrp,
            tc.tile_pool(name="er", bufs=3) as erp,
            tc.tile_pool(name="pvsb", bufs=2) as pvsb,
            tc.tile_pool(name="oall", bufs=2) as oallp,
            tc.tile_pool(name="rz", bufs=4) as rzp,
            tc.tile_pool(name="sps", bufs=2, space="PSUM") as sps,
            tc.tile_pool(name="pvps", bufs=1, space="PSUM") as pvps,
            tc.tile_pool(name="trps", bufs=2, space="PSUM") as trps,
        ):
            ident = consts.tile([128, 128], f32)
            make_identity(nc, ident)
            identh = consts.tile([128, 128], fp16)
            nc.vector.tensor_copy(out=identh, in_=ident)
            wsrc = consts.tile([128, 512], fp16)
            nc.vector.memset(wsrc, 1.0)

            def warmer(n=512):
                wt = trps.tile([64, 512], f32, tag="tr")
                nc.tensor.matmul(wt[:, 0:n], wsrc[:, 0:64], wsrc[:, 0:n],
                                 start=True, stop=True, skip_group_check=True)

            # per-batch persistent tiles
            qkT = {}   # b -> (qhT, khT) [128, 2048] fp16, halves identical
            v_r = {}   # b -> [128, 16, 65] fp16  (col 64 = 1.0)

            def prep_load(b):
                """DMA loads + fp16 casts + transpose jobs for batch b."""
                q_nat = nat.tile([128, LT, D], f32, tag="qnat")
                k_nat = nat.tile([128, LT, D], f32, tag="knat")
                nc.sync.dma_start(
                    out=q_nat, in_=q_d.ap()[b].rearrange("(t p) d -> p t d", p=128))
                nc.sync.dma_start(
                    out=k_nat, in_=k_d.ap()[b].rearrange("(t p) d -> p t d", p=128))

                qh_nat = nath.tile([128, LT, D], fp16, tag="qh_nat")
                kh_nat = nath.tile([128, LT, D], fp16, tag="kh_nat")
                nc.vector.tensor_copy(out=qh_nat, in_=q_nat)
                nc.vector.tensor_copy(out=kh_nat, in_=k_nat)

                qhT = qkt.tile([128, L], fp16, tag="qhT")
                khT = qkt.tile([128, L], fp16, tag="khT")

                v_stage = vst.tile([128, LT, D], f32, tag="vstage")
                nc.sync.dma_start(
                    out=v_stage, in_=v_d.ap()[b].rearrange("(t p) d -> p t d", p=128))
                vr = vrp.tile([128, LT, D + 1], fp16, tag="vr")
                nc.vector.tensor_copy(out=vr[:, :, 0:D], in_=v_stage)
                nc.vector.memset(vr[:, :, D:D + 1], 1.0)

                qkT[b] = (qhT, khT)
                v_r[b] = vr

                jobs = []
                for lt in range(LT):
                    for src, dst in ((qh_nat, qhT), (kh_nat, khT)):
                        def tr_job(src=src, dst=dst, lt=lt):
                            tp = trps.tile([64, 128], fp16, tag="tr")
                            nc.tensor.transpose(tp, src[:, lt, :], identh)
                            nc.vector.tensor_copy(
                                out=dst[0:64, lt * 128:(lt + 1) * 128], in_=tp)
                        jobs.append(tr_job)

                def dup_job(t):
                    def job():
                        nc.sync.dma_start(out=t[64:128, :], in_=t[0:64, :])
                    return job

                jobs.append(dup_job(qhT))
                jobs.append(dup_job(khT))
                return jobs

            pending = []   # deferred small jobs woven into the MM stream

            def main(b, next_jobs):
                qhT, khT = qkT.pop(b)
                vr = v_r.pop(b)
                slot = 0
                for qh in range(NQH):
                    q0 = qh * QHW
                    pv = pvps.tile([D + 1, QHW], f32, tag="pv")
                    for kp in range(LT // 2):      # pairs of k-tiles
                        ka, kb = 2 * kp, 2 * kp + 1
                        # interleave deferred out-work + next batch's prep
                        if pending:
                            pending.pop(0)()
                        for _ in range(2):
                            if slot < len(next_jobs):
                                next_jobs[slot]()
                                slot += 1
                        s_a = sps.tile([128, QHW], f32, tag="s")
                        s_b = sps.tile([128, QHW], f32, tag="s")
                        ha, hb = slice(0, 64), slice(64, 128)
                        ksa = slice(ka * 128, (ka + 1) * 128)
                        ksb = slice(kb * 128, (kb + 1) * 128)
                        for j in range(QHW // 512):
                            js = slice(j * 512, (j + 1) * 512)
                            qs = slice(q0 + j * 512, q0 + (j + 1) * 512)
                            nc.tensor.matmul(
                                s_a[:, js], khT[ha, ksa], qhT[ha, qs],
                                start=True, stop=True)
                            nc.tensor.matmul(
                                s_b[:, js], khT[hb, ksb], qhT[hb, qs],
                                start=True, stop=True)
                        for kt, s_ps in ((ka, s_a), (kb, s_b)):
                            e_r = erp.tile([128, QHW], fp16, tag="e")
                            nc.scalar.activation(out=e_r, in_=s_ps, func=Exp,
                                                 scale=float(SCALE))
                            for j in range(QHW // 512):
                                js = slice(j * 512, (j + 1) * 512)
                                nc.tensor.matmul(
                                    pv[:, js], vr[:, kt, :], e_r[:, js],
                                    start=(kt == 0), stop=(kt == LT - 1))

                    # defer psum evacuation + out-transpose + normalize:
                    # woven into subsequent pair-slots so the in-order PE
                    # stream never blocks on this at the qh boundary
                    pv_sb = pvsb.tile([D + 1, QHW], f32, tag="pvsb")
                    o_all = oallp.tile([128, QHW // 128, D], f32, tag="oall")

                    def evac_job(pv=pv, pv_sb=pv_sb):
                        nc.vector.tensor_copy(out=pv_sb, in_=pv)
                    pending.append(evac_job)

                    for qt in range(QHW // 128):
                        def out_job(qt=qt, pv_sb=pv_sb, o_all=o_all):
                            ot = trps.tile([128, D + 1], f32, tag="tr")
                            nc.tensor.transpose(
                                ot, pv_sb[:, qt * 128:(qt + 1) * 128],
                                ident[0:D + 1, 0:D + 1])
                            rz = rzp.tile([128, 1], f32, tag="rz")
                            nc.vector.reciprocal(out=rz, in_=ot[:, D:D + 1])
                            nc.vector.tensor_scalar_mul(
                                out=o_all[:, qt, :], in0=ot[:, 0:D],
                                scalar1=rz)
                        pending.append(out_job)

                    def store_job(b=b, q0=q0, o_all=o_all):
                        nc.sync.dma_start(
                            out=o_d.ap()[b, q0:q0 + QHW, :].rearrange(
                                "(t p) d -> p t d", p=128),
                            in_=o_all)
                    pending.append(store_job)
                while slot < len(next_jobs):
                    next_jobs[slot]()
                    slot += 1

            for _ in range(24):
                warmer()
            jobs0 = prep_load(0)
            for job in jobs0:
                job()
            for b in range(B_SH):
                nxt = prep_load(b + 1) if b + 1 < B_SH else []
                main(b, nxt)
            for job in pending:
                job()

    nc.finalize()
    return nc


def _get_nc():
    if "nc" not in _cached:
        _cached["nc"] = _build()
    return _cached["nc"]


def kernel(query, key, value):
    from concourse.bass_utils import run_bass_kernel_spmd

    nc = _get_nc()
    query = np.ascontiguousarray(query, dtype=np.float32)
    key = np.ascontiguousarray(key, dtype=np.float32)
    value = np.ascontiguousarray(value, dtype=np.float32)

    in_maps = []
    for c in range(NCORES):
        sl = slice(c * B_SH, (c + 1) * B_SH)
        in_maps.append({
            "query": query[sl], "key": key[sl], "value": value[sl]})

    res = run_bass_kernel_spmd(nc, in_maps, core_ids=list(range(NCORES)))
    out = np.concatenate([r["out"] for r in res.results], axis=0)
    return out
